# revision 3
# baseline (speedup 1.0000x reference)
"""Trainium2 Bass kernel for nn_DetectorLossFn (detector loss with IoU argmax).

v2 strategy
-----------
Data-parallel over N=16 across 8 cores, with BOTH of a core's 2 batches packed
into one 128-partition tile set: partitions 0-63 hold batch A (pred chunk
p covers k = p*512 + b, b in [0,512)), partitions 64-127 hold batch B.  All
inner-loop ops are [128, 512].

Ranking surrogate: argmax_k iou(m,k) == argmax_k inter(m,k) / (relu(a1) + a2)
  (for any candidate with inter>0, a1>0 and den = a1+a2-inter >= max(a1,a2) > 0,
   so the cross-multiplied ranking is identical; inter==0 candidates give
   r = iou = 0.  Verified exact argmax match on the real inputs in fp32.)

This removes `inter` from the denominator, so the reciprocal no longer
depends on the DVE side computation and moves to the Scalar engine as
recip(v) = exp(-ln(v)) (one table set holds both ln and exp), with
v = A1C + a2[m] computed inside the Ln activation's per-partition bias.

Per target m (128 iterations), engines split:
  ACT : LV = ln(A1C + a2[m]);  RV = exp(-LV)            (2 ops)
  DVE : DX = min(PX2P, tx2p) - max(PX1, tx1)            (custom, 1 op)
        DY = min(PY2P, ty2p) - max(PY1, ty1)            (custom, 1 op)
        IOU = relu(DX)*relu(DYR), lanemax -> LMAX[:,m]  (custom, 1 op)
        SCR = (IOU==lanemax)*ENC, lanemax -> LENC[:,m]  (custom, 1 op)
  POOL: DYR = DY * RV                                   (1 op)

(The "+1" box-side offsets are folded into host-precomputed PX2P/TX2P, and
relu-clamping of the sides is folded into the IOU multiply: relu(dy*rv) =
relu(dy)*rv since rv>0.)

A small cross-partition finale (PE transpose + per-batch-half masked reduce)
produces, per (n, m), enc = 32768 - argmax_k.  Host decodes indices and runs
the cheap O(N*M*C) loss epilogue in float32 exactly mirroring the reference.
"""

import sys

import numpy as np

for _p in ("/opt/trn_rl_repo",):
    if _p not in sys.path:
        sys.path.insert(0, _p)

import concourse.bass as bass
import concourse.bacc as bacc
import concourse.mybir as mybir
from concourse.bass_utils import run_bass_kernel_spmd
from concourse.tile import TileContext
from concourse import dve_ops
from concourse.dve_spec import (
    C0,
    C1,
    Spec,
    Src0,
    Src1,
    Zero,
    _has_src1,
    eq,
    lower,
    maxx,
    minn,
    relu,
)
from concourse.dve_uop import DveOpSpec

F32 = mybir.dt.float32
ALU = mybir.AluOpType
AF = mybir.ActivationFunctionType

N, K, C, M = 16, 32768, 16, 128
NCORES = 8
_COMBINED_ACT_SET_ID = 6   # natural_log_exp_and_others in act_info.json
NB = N // NCORES   # batches per core (2)
P = 128            # SBUF partitions
HP = P // NB       # partitions per batch (64)
Q = K // HP        # free-dim length per lane (512)


# --------------------------------------------------------------------------
# Custom DVE ops
# --------------------------------------------------------------------------
def _register(name, spec, subdim=False):
    for op in dve_ops.OPS:
        if op.name == name:
            return op
    probe = dve_ops.DveOp(name, spec, subdim, uops_sha={})
    dve_ops.OPS.append(probe)
    dve_ops._SUB_OPCODE_FOR_NAME[name] = (
        dve_ops._CUSTOM_DVE_ROW_BASE + len(dve_ops.OPS) - 1)
    assert dve_ops._SUB_OPCODE_FOR_NAME[name] < 0x20
    opcode = dve_ops.get_dve_sub_opcode(name)
    shas = {}
    for ver in ("v3", "v4"):
        s = DveOpSpec(
            name=name, opcode=opcode, uops=lower(spec, ver=ver),
            rd1_en=_has_src1(spec),
        )
        shas[ver] = s.sha(ver)
    real = dve_ops.DveOp(name, spec, subdim, uops_sha=shas)
    dve_ops.OPS[dve_ops.OPS.index(probe)] = real
    dve_ops.CUSTOM_DVE_SPECS[name] = spec
    return real


def _ref_side(in0, in1, s0, s1, imm2):
    return (np.minimum(in0, s1) - np.maximum(in1, s0)).astype(np.float32)


def _ref_relmulmax(in0, in1, s0, s1, imm2):
    b = (np.maximum(in0, np.float32(0))
         * np.maximum(in1, np.float32(0))).astype(np.float32)
    acc = b.reshape(b.shape[0], -1).max(axis=-1, keepdims=True)
    return b, np.maximum(acc, np.float32(-3.4028235e38))


def _ref_eqenc(in0, in1, s0, s1, imm2):
    b = ((in0 == s0).astype(np.float32) * in1).astype(np.float32)
    acc = b.reshape(b.shape[0], -1).max(axis=-1, keepdims=True)
    return b, np.maximum(acc, np.float32(0))


# dxu = min(px2p, tx2p) - max(px1, tx1)   (unclamped side; +1 pre-folded)
SIDE_OP = _register(
    "ANT_IOU2_SIDE",
    Spec(body=minn(Src0, C1) - maxx(Src1, C0), reference=_ref_side),
)
# iou = relu(dxu) * relu(dyru) ; accum_out = lane max
RELMULMAX_OP = _register(
    "ANT_IOU2_RELMULMAX",
    Spec(body=relu(Src0) * relu(Src1), accum=maxx, reference=_ref_relmulmax),
)
# lane-argmax encode: out = (iou == lanemax) * enc ; accum_out = max(out, 0)
EQENC_OP = _register(
    "ANT_IOU2_EQENC",
    Spec(body=eq(Src0, C0) * Src1, accum=maxx, accum_init=Zero,
         reference=_ref_eqenc),
)


# --------------------------------------------------------------------------
# Device kernel builder
# --------------------------------------------------------------------------
def build_nc(nb=NB, q=Q, reps=1, variant=""):
    """Build the per-core Bass program (identical on all cores; SPMD).

    reps > 1 re-emits the whole workload serially (for slope-based timing).
    variant: comma-separated timing-experiment flags (NOT for correctness):
      "dyrv"   — DYR multiply on vector engine instead of gpsimd
      "noeq"   — drop the EQENC op (argmax indices wrong)
      "noact"  — drop the ACT ln/exp chain, RV := A1C (iou values wrong)
    """
    vflags = set(v for v in variant.split(",") if v)
    nc = bacc.Bacc("TRN2", target_bir_lowering=False)

    # All activations in this kernel (Ln, Exp, Copy) are served by one
    # table set, natural_log_exp_and_others (act_func_set_id 6 in
    # act_info.json).  The stock insert_act_table_loads pass picks the
    # first set containing each func (natural_log for Ln, exp_and_others
    # for Exp), which ping-pongs a ~1.3us table load around every
    # activation.  Pre-place a single combined-set load at program start
    # instead (instance-level override; no global state touched).
    def _load_combined_act_table():
        has_activation = any(
            isinstance(i, mybir.InstActivation)
            for b in nc.main_func.blocks
            for i in b.instructions
        )
        if not has_activation:
            return
        load = mybir.InstLoadActFuncSet(
            name=nc.get_next_instruction_name(), ins=[], outs=[],
            act_func_set_id=_COMBINED_ACT_SET_ID)
        load.engine = mybir.EngineType.Activation
        nc.register_instruction(load)
        nc.main_func.blocks[0].instructions.insert(0, load)

    nc.insert_act_table_loads = _load_combined_act_table

    # [0:q] PX1 | [q:2q] PY1 | [2q:3q] PX2P | [3q:4q] PY2P | [4q:5q] A1C
    pbt_d = nc.declare_dram_parameter("pbt", [P, 5 * q], F32, isOutput=False)
    # [0:M] TX1 | [M:2M] TY1 | [2M:3M] TX2P | [3M:4M] TY2P | [4M:5M] A2
    tgt_d = nc.declare_dram_parameter("tgt", [P, 5 * M], F32, isOutput=False)
    # [0:q] ENC | [q:q+M] PROWF | [q+M:q+2M] IDENT
    cst_d = nc.declare_dram_parameter("cst", [P, q + 2 * M], F32,
                                      isOutput=False)
    oenc_d = nc.declare_dram_parameter("oenc", [M, NB], F32, isOutput=True)

    with TileContext(nc) as tc:
        with (
            tc.tile_pool(name="const", bufs=1) as cpool,
            tc.tile_pool(name="batch", bufs=2) as bpool,
            tc.tile_pool(name="lns", bufs=9) as lpool,
            tc.tile_pool(name="work", bufs=3) as wpool,
            tc.tile_pool(name="fin", bufs=2) as fpool,
            tc.tile_pool(name="psum", bufs=2, space="PSUM") as ppool,
        ):
            CST = cpool.tile([P, q + 2 * M], F32, tag="CST")
            nc.sync.dma_start(out=CST[:], in_=cst_d[:, :])
            ENC = CST[:, 0:q]
            PROWF = CST[:, q:q + M]
            IDENT = CST[:, q + M:q + 2 * M]
            OUTS = cpool.tile([P, NB], F32, tag="OUTS")
            OM = cpool.tile([P, NB], F32, tag="OM")

            for _ in range(reps):
                PBT = bpool.tile([P, 5 * q], F32, tag="PBT")
                nc.sync.dma_start(out=PBT[:], in_=pbt_d[:, :])
                PX1 = PBT[:, 0:q]
                PY1 = PBT[:, q:2 * q]
                PX2P = PBT[:, 2 * q:3 * q]
                PY2P = PBT[:, 3 * q:4 * q]
                A1C = PBT[:, 4 * q:5 * q]

                TGT = bpool.tile([P, 5 * M], F32, tag="TGT")
                nc.sync.dma_start(out=TGT[:], in_=tgt_d[:, :])
                TX1 = TGT[:, 0:M]
                TY1 = TGT[:, M:2 * M]
                TX2P = TGT[:, 2 * M:3 * M]
                TY2P = TGT[:, 3 * M:4 * M]
                A2 = TGT[:, 4 * M:5 * M]

                LMAX = bpool.tile([P, M], F32, tag="LMAX")
                LENC = bpool.tile([P, M], F32, tag="LENC")

                # ---- main loop over targets, in ACT-table-friendly blocks:
                # per block of B targets run all Ln's, then all Exp's (2
                # table loads per block instead of 2 per target), then the
                # per-target DVE/Pool chain consuming the blocked RV tiles.
                B = 8
                for blk in range(M // B):
                    rvs = []
                    if "noact" in vflags:
                        for j in range(B):
                            RV = lpool.tile([P, q], F32, tag="RV")
                            nc.scalar.copy(RV[:], A1C)
                            rvs.append(RV)
                    else:
                        lvs = []
                        for j in range(B):
                            m = blk * B + j
                            LV = lpool.tile([P, q], F32, tag="LV")
                            nc.scalar.activation(
                                LV[:], A1C, AF.Ln,
                                bias=A2[:, m:m + 1], scale=1.0)
                            lvs.append(LV)
                        for j in range(B):
                            RV = lpool.tile([P, q], F32, tag="RV", bufs=12)
                            nc.scalar.activation(
                                RV[:], lvs[j][:], AF.Exp, bias=0.0,
                                scale=-1.0)
                            rvs.append(RV)
                    for j in range(B):
                        m = blk * B + j
                        DY = wpool.tile([P, q], F32, tag="DY", bufs=4)
                        nc.vector._custom_dve(
                            SIDE_OP, out=DY[:], in0=PY2P, in1=PY1,
                            s0=TY1[:, m:m + 1], s1=TY2P[:, m:m + 1])
                        DX = wpool.tile([P, q], F32, tag="DX", bufs=4)
                        nc.vector._custom_dve(
                            SIDE_OP, out=DX[:], in0=PX2P, in1=PX1,
                            s0=TX1[:, m:m + 1], s1=TX2P[:, m:m + 1])
                        DYR = wpool.tile([P, q], F32, tag="DYR", bufs=4)
                        if "dyrv" in vflags:
                            nc.vector.tensor_tensor(DYR[:], DY[:], rvs[j][:],
                                                    ALU.mult)
                        else:
                            nc.gpsimd.tensor_tensor(DYR[:], DY[:], rvs[j][:],
                                                    ALU.mult)
                        IOU = wpool.tile([P, q], F32, tag="IOU")
                        nc.vector._custom_dve(
                            RELMULMAX_OP, out=IOU[:], in0=DX[:], in1=DYR[:],
                            accum_out=LMAX[:, m:m + 1])
                        if "noeq" not in vflags:
                            SCR = wpool.tile([P, q], F32, tag="SCR")
                            nc.vector._custom_dve(
                                EQENC_OP, out=SCR[:], in0=IOU[:], in1=ENC,
                                s0=LMAX[:, m:m + 1],
                                accum_out=LENC[:, m:m + 1])

                # ---- cross-partition finale -----------------------------
                ptm = ppool.tile([P, M], F32, tag="ptm")
                nc.tensor.transpose(ptm[:], LMAX[:], IDENT)
                LMAXT = fpool.tile([P, M], F32, tag="LMAXT")
                nc.scalar.copy(LMAXT[:], ptm[:])
                pte = ppool.tile([P, M], F32, tag="pte")
                nc.tensor.transpose(pte[:], LENC[:], IDENT)
                LENCT = fpool.tile([P, M], F32, tag="LENCT")
                nc.scalar.copy(LENCT[:], pte[:])
                T1 = fpool.tile([P, M], F32, tag="T1")
                nc.gpsimd.tensor_tensor(T1[:], LENCT[:], PROWF, ALU.add)
                for n in range(NB):
                    sl = slice(n * HP, (n + 1) * HP)
                    nc.vector.tensor_reduce(
                        OM[:, n:n + 1], LMAXT[:, sl],
                        axis=mybir.AxisListType.X, op=ALU.max)
                    msk = fpool.tile([P, HP], F32, tag=f"msk{n}")
                    nc.vector.tensor_scalar(
                        msk[:], LMAXT[:, sl], OM[:, n:n + 1], None,
                        ALU.is_equal)
                    t2 = fpool.tile([P, HP], F32, tag=f"t2{n}")
                    nc.gpsimd.tensor_tensor(t2[:], msk[:], T1[:, sl], ALU.mult)
                    nc.vector.tensor_reduce(
                        OUTS[:, n:n + 1], t2[:],
                        axis=mybir.AxisListType.X, op=ALU.max)
                nc.sync.dma_start(out=oenc_d[:, :], in_=OUTS[:])
    nc.finalize()
    return nc


# --------------------------------------------------------------------------
# Host-side input prep, device run, epilogue
# --------------------------------------------------------------------------
def _make_in_maps(pred_boxes, target, nb=NB, q=Q, ncores=NCORES):
    f32 = np.float32
    one = f32(1)
    enc = np.broadcast_to((q - np.arange(q, dtype=f32))[None, :], (P, q))
    prowf = np.broadcast_to(
        (q * (HP - 1 - (np.arange(P, dtype=f32) % HP)))[None, :], (P, M))
    ident = np.eye(P, dtype=f32)
    cst = np.concatenate(
        [enc, prowf[:, :M], ident], axis=1).astype(f32)
    cst = np.ascontiguousarray(cst)

    in_maps = []
    for c in range(ncores):
        pbt = np.empty((P, 5 * q), dtype=f32)
        tgt = np.empty((P, 5 * M), dtype=f32)
        for n in range(nb):
            rows = slice(n * HP, (n + 1) * HP)
            arr = pred_boxes[c * nb + n]          # [K, 5] f32
            x = arr[:, 0].reshape(HP, q)
            y = arr[:, 1].reshape(HP, q)
            w = arr[:, 2].reshape(HP, q)
            h = arr[:, 3].reshape(HP, q)
            pbt[rows, 0:q] = x
            pbt[rows, q:2 * q] = y
            pbt[rows, 2 * q:3 * q] = (x + w) + one
            pbt[rows, 3 * q:4 * q] = (y + h) + one
            pbt[rows, 4 * q:5 * q] = np.maximum(
                ((w + one) * (h + one)).astype(f32), f32(0))

            t = target[c * nb + n]                # [M, 5] f32
            tx1 = t[:, 1]
            ty1 = t[:, 2]
            tx2 = t[:, 3]
            ty2 = t[:, 4]
            tgt[rows, 0:M] = tx1[None, :]
            tgt[rows, M:2 * M] = ty1[None, :]
            tgt[rows, 2 * M:3 * M] = (tx2 + one)[None, :]
            tgt[rows, 3 * M:4 * M] = (ty2 + one)[None, :]
            tgt[rows, 4 * M:5 * M] = (
                ((tx2 - tx1) + one) * ((ty2 - ty1) + one)).astype(f32)[None, :]
        in_maps.append({"pbt": pbt, "tgt": tgt, "cst": cst})
    return in_maps


def _epilogue(pred_boxes, pred_cls, target, best):
    """Numpy float32 replica of the reference loss math, given argmax picks."""
    f32 = np.float32
    n_, k_, _ = pred_boxes.shape
    pb = pred_boxes[..., :4].astype(f32)
    mask = target.sum(axis=2) != 0
    maskf = mask.astype(f32)
    denom = maskf.sum(dtype=f32)
    tboxes = target[..., 1:].astype(f32)
    tcls = np.clip(target[..., 0].astype(np.int32), 0, pred_cls.shape[2] - 1)
    best_idx = np.where(mask, best, 0)
    ar = np.arange(n_)[:, None]
    best_pb = pb[ar, best_idx]
    best_cls = pred_cls[ar, best_idx].astype(f32)
    pconf = pred_boxes[..., 4].astype(f32)
    best_conf = (1.0 / (1.0 + np.exp(-pconf[:, 0:1], dtype=f32))).astype(f32)
    best_conf = np.broadcast_to(best_conf, mask.shape).astype(f32)

    def masked_mean(v):
        return (v.astype(f32) * maskf).sum(dtype=f32) / denom

    mx = best_cls.max(axis=-1, keepdims=True)
    lse = np.log(np.exp(best_cls - mx).sum(axis=-1, keepdims=True)) + mx
    logp = best_cls - lse
    ce = -np.take_along_axis(logp, tcls[..., None], axis=-1)[..., 0]
    loss_cls = masked_mean(ce)
    loss_x = masked_mean((best_pb[..., 0] - tboxes[..., 0]) ** 2)
    loss_y = masked_mean((best_pb[..., 1] - tboxes[..., 1]) ** 2)
    loss_w = masked_mean((best_pb[..., 2] - (tboxes[..., 2] - tboxes[..., 0])) ** 2)
    loss_h = masked_mean((best_pb[..., 3] - (tboxes[..., 3] - tboxes[..., 1])) ** 2)
    labels = (best_conf > 0.5).astype(f32)
    bce = -(labels * np.log(best_conf) +
            (1.0 - labels) * np.log(1.0 - best_conf))
    loss_conf = masked_mean(bce)
    loss = f32(loss_cls + loss_x + loss_y + loss_w + loss_h + loss_conf)
    return (loss, f32(loss_cls), f32(loss_x), f32(loss_y), f32(loss_w),
            f32(loss_h), f32(loss_conf))


_NC_CACHE = {}


def _get_nc():
    key = (NB, Q)
    if key not in _NC_CACHE:
        _NC_CACHE[key] = build_nc(NB, Q)
    return _NC_CACHE[key]


def _decode(results):
    """oenc [M, NB] per core -> best [N, M] int64 (k = K - enc)."""
    best = np.zeros((N, M), dtype=np.int64)
    for c in range(NCORES):
        enc = results[c]["oenc"]                  # [M, NB]
        kk = np.float64(K) - enc.T                # [NB, M]
        best[c * NB:(c + 1) * NB] = np.clip(
            np.rint(kk).astype(np.int64), 0, K - 1)
    return best


def run_device(pred_boxes, target, trace=False):
    """Run the Bass kernel on 8 cores; returns (best[N, M] int64, results)."""
    nc = _get_nc()
    in_maps = _make_in_maps(pred_boxes, target)
    res = run_bass_kernel_spmd(nc, in_maps, list(range(NCORES)), trace=trace)
    return _decode(res.results), res


def kernel(pred_boxes, pred_cls, target):
    pred_boxes = np.asarray(pred_boxes, dtype=np.float32)
    pred_cls = np.asarray(pred_cls, dtype=np.float32)
    target = np.asarray(target, dtype=np.float32)
    best, _ = run_device(pred_boxes, target)
    return _epilogue(pred_boxes, pred_cls, target, best)
